# revision 7
# baseline (speedup 1.0000x reference)
"""Trainium2 Bass kernel v3: MultiHeadAttention with QK-RMSNorm + partial
rotary, causal softmax. B=4, T=2048, D=1024, H=16, HD=64, fp32 in/out.

Sharding: 8 cores = 4 batches x 2 head-groups (8 heads each per core).
Host sums the two head-group partials per batch and transposes back.

v3 changes vs baseline (all aimed at engine overlap + busy-time cuts):
  - Cross-phase software pipeline: QKV proj / attention / out-proj are
    emitted interleaved (tt0,tt1, attn-i[0:1024), oproj-t0/t1, tt2,tt3,
    attn-i[1024:2048), oproj-t2/t3) so ACT(exp) overlaps proj/oproj PE.
  - Q/K/V post-norm tensors, exp(P), otf and wo are bf16 (scores/AV/
    out-proj matmuls in bf16): halves SBUF, enables the pipeline to fit.
  - K is normalized directly (like Q) instead of folding its norm into
    exp's scale: kills the end-of-phase-1 nkcols barrier + 128 tiny MMs.
  - rsqrt via Ln+Exp (one ACT table set with softmax's Exp - no table
    thrash); squares on GPSIMD not ACT; phase-3 PSUM drain on DVE.
  - Causal trim: scores/exp/AV instructions only cover i >= j_chunk
    (saves ~25k PE cycles + the zero-fill of P gaps).
  - attention processed in 512-wide i-quarters: PSUM fits proj(2) +
    scratch(2) + scores(2) + AV-accum(2) banks concurrently.
"""

import numpy as np
from contextlib import ExitStack

import concourse.bass as bass
import concourse.tile as tile
import concourse.mybir as mybir
from concourse import bacc
from concourse.hw_specs import get_activation_tables

F32 = mybir.dt.float32
BF16 = mybir.dt.bfloat16
MM_DT = mybir.dt.float32r
AF = mybir.ActivationFunctionType

D = 1024   # model dim
DH = 512   # head-group width per core (8 heads x 64)
NH = 8     # heads per core
HD = 64    # head dim
NKC = D // 128   # k-chunks over model dim
EPS = 1e-6


def _r(ap):
    return ap.bitcast(MM_DT)


def build_kernel(nc: bass.Bass, T: int = 2048):
    NTT = T // 512          # 512-wide t/i blocks
    NTS = T // 128          # 128-wide j chunks
    HW = min(1024, T)       # half width (q/k tile column span)
    NHALF = T // HW

    xt = nc.dram_tensor("xt", [D, T], BF16, kind="ExternalInput").ap()
    wqt = nc.dram_tensor("wqt", [D, DH], BF16, kind="ExternalInput").ap()
    wkt = nc.dram_tensor("wkt", [D, DH], BF16, kind="ExternalInput").ap()
    wvt = nc.dram_tensor("wvt", [D, DH], BF16, kind="ExternalInput").ap()
    wob = nc.dram_tensor("wob", [DH, D], BF16, kind="ExternalInput").ap()
    c2d = nc.dram_tensor("c2", [128, T], BF16, kind="ExternalInput").ap()
    s2d = nc.dram_tensor("s2", [128, T], BF16, kind="ExternalInput").ap()
    pswapd = nc.dram_tensor("pswap", [128, 128], F32, kind="ExternalInput").ap()
    bdiagd = nc.dram_tensor("bdiag", [128, 128], F32, kind="ExternalInput").ap()
    trid = nc.dram_tensor("tri", [128, 128], BF16, kind="ExternalInput").ap()
    yt = nc.dram_tensor("yt", [D, T], F32, kind="ExternalOutput").ap()

    # Pin the ACT table to the one set serving Ln AND Exp so bacc's
    # greedy table-load pass doesn't thrash between exp/ln sets.
    act_set_id = list(get_activation_tables(nc.m.arch)).index(
        "natural_log_exp_and_others")

    with tile.TileContext(nc) as tc, ExitStack() as ctx:
        # ---------------- pools (flat; all coexist) ---------------------
        cst = ctx.enter_context(tc.tile_pool(name="cst", bufs=1))
        qk_pool = ctx.enter_context(tc.tile_pool(name="qk", bufs=1))
        v_pool = ctx.enter_context(tc.tile_pool(name="v", bufs=1))
        o_pool = ctx.enter_context(tc.tile_pool(name="ot", bufs=1))
        w_pool = ctx.enter_context(tc.tile_pool(name="w", bufs=1))
        rc_pool = ctx.enter_context(tc.tile_pool(name="rc", bufs=1))
        x_pool = ctx.enter_context(tc.tile_pool(name="xs", bufs=5))
        sg_pool = ctx.enter_context(tc.tile_pool(name="sg", bufs=2))
        p_pool = ctx.enter_context(tc.tile_pool(name="pe", bufs=4))
        st_pool = ctx.enter_context(tc.tile_pool(name="st", bufs=3))
        e_pool = ctx.enter_context(tc.tile_pool(name="ep", bufs=2))
        # PSUM: 8 banks = scratch(2: proj/pswap/sumsq/oproj share one tag)
        # + scores(4: [128,1024] double-buffered) + AV accumulators(2)
        ps_scr = ctx.enter_context(tc.tile_pool(name="scr", bufs=2, space="PSUM"))
        ps_sc = ctx.enter_context(tc.tile_pool(name="sc", bufs=2, space="PSUM"))
        ps_ot = ctx.enter_context(tc.tile_pool(name="po", bufs=1, space="PSUM"))
        ps_pp = ps_scr  # alias kept for readability

        # ---------------- persistent tiles ------------------------------
        pswap = cst.tile([128, 128], F32, name="pswap_s")
        bdiag = cst.tile([128, 128], F32, name="bdiag_s")
        tri = cst.tile([128, 128], BF16, name="tri_s")
        epsb = cst.tile([128, 1], F32, name="epsb")
        # q/k: per head-pair-tile, per half: [128, HW] bf16 (normed+rotated)
        qt_s = [[qk_pool.tile([128, HW], BF16, name=f"qt{j}_{h}")
                 for h in range(NHALF)] for j in range(4)]
        kt_s = [[qk_pool.tile([128, HW], BF16, name=f"kt{j}_{h}")
                 for h in range(NHALF)] for j in range(4)]
        # V: [128, 8*128] bf16 per 128-token chunk. Per-head 128 cols =
        # [ones, 63 zeros, V 64] so the AV psum puts the softmax
        # denominator at row 0 and the payload at rows 64:128 - legal
        # base partitions (0 / 64) for the epilogue reads.
        v_s = [v_pool.tile([128, NH * 128], BF16, name=f"vt{j}")
               for j in range(NTS)]
        # attention out (normalized): 4 tiles [128, T] bf16
        otf = [o_pool.tile([128, T], BF16, name=f"otf{j}") for j in range(4)]
        wq_s = [w_pool.tile([128, DH], BF16, name=f"wq{k}") for k in range(NKC)]
        wk_s = [w_pool.tile([128, DH], BF16, name=f"wk{k}") for k in range(NKC)]
        wv_s = [w_pool.tile([128, DH], BF16, name=f"wv{k}") for k in range(NKC)]
        wo_s = [w_pool.tile([128, D], BF16, name=f"wo{k}") for k in range(4)]
        c2 = [rc_pool.tile([128, HW], BF16, name=f"c2_{h}")
              for h in range(NHALF)]
        s2 = [rc_pool.tile([128, HW], BF16, name=f"s2_{h}")
              for h in range(NHALF)]

        # ---------------- input DMAs ------------------------------------
        for k in range(NKC):
            ksl = slice(k * 128, (k + 1) * 128)
            nc.sync.dma_start(wq_s[k][:], wqt[ksl, :])
        nc.sync.dma_start(_r(pswap[:]), _r(pswapd[:]))
        nc.sync.dma_start(_r(bdiag[:]), _r(bdiagd[:]))
        nc.sync.dma_start(tri[:], trid[:])
        for k in range(NKC):
            ksl = slice(k * 128, (k + 1) * 128)
            nc.sync.dma_start(wv_s[k][:], wvt[ksl, :])
        for k4 in range(4):
            nc.sync.dma_start(wo_s[k4][:], wob[k4 * 128:(k4 + 1) * 128, :])
        # rotary tables for half 0 on the (initially idle) ACT queue
        nc.scalar.dma_start(c2[0][:], c2d[:, 0:HW])
        nc.scalar.dma_start(s2[0][:], s2d[:, 0:HW])
        nc.scalar.add_instruction(mybir.InstLoadActFuncSet(
            name=f"I-{nc.next_id()}", ins=[], outs=[],
            act_func_set_id=act_set_id))
        nc.gpsimd.memset(epsb[:], 8.0 * EPS)

        # ---------------- phase bodies ----------------------------------
        def proj_block(tt, jt, wsrc, dst_s):
            """Project + rotate + RMS-normalize one [128, 512] q/k block."""
            w0 = tt < 2  # before attention starts: borrow its idle PSUM
            half, loc = (tt * 512) // HW, (tt * 512) % HW
            lsl = slice(loc, loc + 512)
            pp = (ps_sc if w0 else ps_scr).tile(
                [128, 512], F32, name="pp", tag="sc" if w0 else "scr")
            jsl = slice(jt * 128, (jt + 1) * 128)
            for k in range(NKC):
                nc.tensor.matmul(pp[:], wsrc[k][:, jsl], xc_s[k][:],
                                 start=(k == 0), stop=(k == NKC - 1))
            qw = sg_pool.tile([128, 512], F32, name="qw", tag="qw")
            nc.vector.tensor_copy(_r(qw[:]), pp[:])
            # partition half-swap (d <-> d^32) for the rotary cross term
            xs = (ps_ot if w0 else ps_scr).tile(
                [128, 512], F32, name="xs", tag="po0" if w0 else "scr")
            nc.tensor.matmul(xs[:], _r(pswap[:]), _r(qw[:]),
                             start=True, stop=True)
            # sumsq over each head's 64 dims (rotation preserves norms)
            sq = sg_pool.tile([128, 512], F32, name="sq", tag="sq")
            nc.gpsimd.tensor_mul(_r(sq[:]), qw[:], qw[:])
            ms = (ps_ot if w0 else ps_scr).tile(
                [128, 512], F32, name="ms", tag="po1" if w0 else "scr")
            nc.tensor.matmul(ms[:], _r(bdiag[:]), _r(sq[:]),
                             start=True, stop=True)
            # s1 = (sumsq/8 + 8eps)^-1/2 via exp(-0.5*ln(x)): same ACT
            # table set as softmax's Exp (no table switching)
            s1 = sg_pool.tile([128, 512], F32, name="s1", tag="s1")
            nc.scalar.activation(s1[:], ms[:], AF.Ln, scale=0.125, bias=epsb[:])
            nc.scalar.activation(s1[:], s1[:], AF.Exp, scale=-0.5)
            # rotary: q' = q*c2 + swap(q)*s2, then scale by s1, cast bf16
            nc.gpsimd.tensor_mul(_r(qw[:]), qw[:], c2[half][:, lsl])
            xsb = sg_pool.tile([128, 512], F32, name="xsb", tag="xsb")
            nc.vector.tensor_mul(xsb[:], xs[:], s2[half][:, lsl])
            nc.gpsimd.tensor_add(_r(qw[:]), qw[:], xsb[:])
            nc.gpsimd.tensor_mul(dst_s[jt][half][:, lsl], qw[:], s1[:])

        def v_block(tt, ts):
            ci = tt * 4 + ts
            pv = ps_pp.tile([128, 512], F32, name="pv", tag="scr")
            for k in range(NKC):
                nc.tensor.matmul(
                    pv[:], xc_s[k][:, ts * 128:(ts + 1) * 128],
                    wv_s[k][:], start=(k == 0), stop=(k == NKC - 1))
            v3 = v_s[ci].rearrange("p (h e) -> p h e", h=NH)
            nc.vector.tensor_copy(v3[:, :, 64:128],
                                  pv.rearrange("p (h e) -> p h e", h=NH))

        xc_s = None

        def tt_block(tt):
            nonlocal xc_s
            # two 128-row k-chunks per tile/DMA: halves the SWDGE holds
            # on the Pool queue (its compute was getting starved)
            xc_s = []
            for k2 in range(NKC // 2):
                xc = x_pool.tile([128, 2, 512], BF16, name="xc", tag="xc")
                src_ap = xt[k2 * 256:(k2 + 1) * 256,
                            tt * 512:(tt + 1) * 512]
                nc.gpsimd.dma_start(
                    xc[:],
                    src_ap.rearrange("(a p) f -> p a f", a=2))
                xc_s.append(xc[:, 0, :])
                xc_s.append(xc[:, 1, :])
            if tt == 0:
                # wk rides the Pool DMA queue behind tt0's x chunks: it
                # lands right when tt0's K projections need it, while the
                # SP queue carries wq/wv/wo in parallel.
                for k in range(NKC):
                    nc.gpsimd.dma_start(
                        wk_s[k][:], wkt[k * 128:(k + 1) * 128, :])
                    # (wk stays on the Pool queue: it overlaps the SP
                    # queue's wq/wv stream during the cold ramp)
            for jt in range(4):
                proj_block(tt, jt, wq_s, qt_s)
            for jt in range(4):
                proj_block(tt, jt, wk_s, kt_s)
            for ts in range(4):
                v_block(tt, ts)

        def attn_quarter(iq):
            """Causal attention for i in [512*iq, 512*(iq+1)), all 8 heads."""
            ih, iloc = (iq * 512) // HW, (iq * 512) % HW
            nj = 4 * iq + 4
            for hp in range(4):
                ops = {}
                for h2 in range(2):
                    ops[h2] = ps_ot.tile([128, 512], F32, name="otp",
                                         tag=f"po{h2}")
                for jt in range(nj):
                    jh, jloc = (jt * 128) // HW, (jt * 128) % HW
                    jsl = slice(jloc, jloc + 128)
                    off0 = max(0, 128 * (jt - 4 * iq))  # diag offset
                    sc = ps_sc.tile([128, 1024], F32, name="sc", tag="sc")
                    for h2 in range(2):
                        ho = h2 * 64
                        nc.tensor.matmul(
                            sc[:, h2 * 512 + off0:h2 * 512 + 512],
                            kt_s[hp][jh][ho:ho + 64, jsl],
                            qt_s[hp][ih][ho:ho + 64,
                                         iloc + off0:iloc + 512],
                            start=True, stop=True)
                    # one exp instruction covers both heads (strided AP)
                    p = p_pool.tile([128, 1024], BF16, name="p", tag="p")
                    sc3 = sc.rearrange("p (g c) -> p g c", g=2)
                    p3 = p.rearrange("p (g c) -> p g c", g=2)
                    nc.scalar.activation(p3[:, :, off0:512],
                                         sc3[:, :, off0:512], AF.Exp)
                    if jt >= 4 * iq:  # diagonal chunk: causal mask
                        for h2 in range(2):
                            nc.gpsimd.tensor_mul(
                                p[:, h2 * 512 + off0:h2 * 512 + off0 + 128],
                                p[:, h2 * 512 + off0:h2 * 512 + off0 + 128],
                                tri[:])
                    for h2 in range(2):
                        h = 2 * hp + h2
                        nc.tensor.matmul(
                            ops[h2][:, off0:512],
                            v_s[jt][:, 128 * h:128 * h + 128],
                            p[:, h2 * 512 + off0:h2 * 512 + 512],
                            start=(jt == 0), stop=(jt == nj - 1))
                # epilogue: rows 1..64 / row 0 (softmax denominator).
                # One fast copy drains the PSUM accumulator so the next
                # head-pair's AV chain can start while we divide from SBUF.
                for h2 in range(2):
                    ho = h2 * 64
                    rden = e_pool.tile([1, 512], F32, name="rden", tag="rden")
                    nc.vector.reciprocal_approx_fast(out=rden[:],
                                                     in_=ops[h2][0:1, :])
                    rb = e_pool.tile([64, 512], F32, name="rb", tag="rb")
                    nc.gpsimd.partition_broadcast(rb[:], rden[:], channels=64)
                    nc.vector.tensor_mul(
                        otf[hp][ho:ho + 64, iq * 512:(iq + 1) * 512],
                        ops[h2][64:128, :], rb[:])

        def oproj_block(tt):
            tsl = slice(tt * 512, (tt + 1) * 512)
            for dt_ in range(8):
                dsl = slice(dt_ * 128, (dt_ + 1) * 128)
                py = ps_scr.tile([128, 512], F32, name="py", tag="scr")
                for k4 in range(4):
                    nc.tensor.matmul(py[:], wo_s[k4][:, dsl],
                                     otf[k4][:, tsl],
                                     start=(k4 == 0), stop=(k4 == 3))
                st = st_pool.tile([128, 512], F32, name="st", tag="st")
                nc.vector.tensor_copy(st[:], py[:])
                nc.sync.dma_start(yt[dsl, tsl], st[:])

        # ---------------- emission (pipelined program order) ------------
        # V ones/zero columns on the DVE, which idles until ~7us
        for j in range(NTS):
            v3 = v_s[j].rearrange("p (h e) -> p h e", h=NH)
            nc.vector.memset(v3[:, :, 0:1], 1.0)
            nc.vector.memset(v3[:, :, 1:64], 0.0)
        tt_block(0)
        if NTT > 1:
            tt_block(1)
        attn_quarter(0)
        if NTT > 2:
            nc.sync.dma_start(c2[1][:], c2d[:, HW:2 * HW])
            nc.sync.dma_start(s2[1][:], s2d[:, HW:2 * HW])
            attn_quarter(1)
            tt_block(2)
            attn_quarter(2)   # needs only tt2's q + j-chunks < 1536
            oproj_block(0)
            tt_block(3)
            attn_quarter(3)
            oproj_block(1)
            oproj_block(2)
            oproj_block(3)
        elif NTT > 1:
            attn_quarter(1)
            oproj_block(0)
            oproj_block(1)
        else:
            oproj_block(0)
    return nc


# ---------------- host-side tables & shard prep -------------------------

def host_tables(T: int = 2048):
    import ml_dtypes
    n = HD // 4
    af = (1.0 / 1024) ** np.linspace(0, 1, n, dtype=np.float32)
    af = np.concatenate([af, np.zeros(n, np.float32)])  # [32]
    theta = np.outer(np.arange(T, dtype=np.float32), af)  # [T, 32]
    cosT = np.cos(theta).T.astype(np.float32)  # [32, T]
    sinT = np.sin(theta).T.astype(np.float32)
    c2 = np.tile(cosT, (4, 1))                              # [128, T]
    s2 = np.tile(np.concatenate([sinT, -sinT], 0), (2, 1))  # [128, T]
    km = np.arange(128)
    pswap = (km[:, None] == (km[None, :] ^ 32)).astype(np.float32)
    bdiag = ((km[:, None] // 64) == (km[None, :] // 64)).astype(np.float32)
    r_ = np.arange(128)[:, None]
    c_ = np.arange(128)[None, :]
    tri = (c_ >= r_).astype(ml_dtypes.bfloat16)
    return {"c2": np.ascontiguousarray(c2.astype(ml_dtypes.bfloat16)),
            "s2": np.ascontiguousarray(s2.astype(ml_dtypes.bfloat16)),
            "pswap": pswap, "bdiag": bdiag, "tri": tri}


def core_inputs(x, wq, wk, wv, wo, core: int, T: int = 2048):
    import ml_dtypes
    b, g = core % 4, core // 4
    sl = slice(g * DH, (g + 1) * DH)
    m = {
        "xt": np.ascontiguousarray(
            np.asarray(x[b]).T.astype(ml_dtypes.bfloat16)),
        "wqt": np.ascontiguousarray(
            np.asarray(wq)[sl, :].T.astype(ml_dtypes.bfloat16)),
        "wkt": np.ascontiguousarray(
            np.asarray(wk)[sl, :].T.astype(ml_dtypes.bfloat16)),
        "wvt": np.ascontiguousarray(
            np.asarray(wv)[sl, :].T.astype(ml_dtypes.bfloat16)),
        "wob": np.ascontiguousarray(
            np.asarray(wo)[:, sl].T.astype(ml_dtypes.bfloat16)),
    }
    m.update(host_tables(T))
    return m


_CACHE = {}


def _get_nc(T: int = 2048):
    key = ("nc", T)
    if key not in _CACHE:
        nc = bacc.Bacc("TRN2", target_bir_lowering=False, debug=False)
        build_kernel(nc, T)
        nc.compile()
        _CACHE[key] = nc
    return _CACHE[key]


def kernel(x, wq, wk, wv, wo, mask=None):
    from concourse import bass_utils
    nc = _get_nc(2048)
    in_maps = [core_inputs(x, wq, wk, wv, wo, c) for c in range(8)]
    res = bass_utils.run_bass_kernel_spmd(nc, in_maps, list(range(8)))
    outs = [np.asarray(res.results[c]["yt"]) for c in range(8)]
    out = np.empty((4, 2048, 1024), np.float32)
    for b in range(4):
        out[b] = (outs[b] + outs[b + 4]).T
    return out


# revision 8
# speedup vs baseline: 1.0469x; 1.0469x over previous
"""Trainium2 Bass kernel v3: MultiHeadAttention with QK-RMSNorm + partial
rotary, causal softmax. B=4, T=2048, D=1024, H=16, HD=64, fp32 in/out.

Sharding: 8 cores = 4 batches x 2 head-groups (8 heads each per core).
Host sums the two head-group partials per batch and transposes back.

v3 changes vs baseline (all aimed at engine overlap + busy-time cuts):
  - Cross-phase software pipeline: QKV proj / attention / out-proj are
    emitted interleaved (tt0,tt1, attn-i[0:1024), oproj-t0/t1, tt2,tt3,
    attn-i[1024:2048), oproj-t2/t3) so ACT(exp) overlaps proj/oproj PE.
  - Q/K/V post-norm tensors, exp(P), otf and wo are bf16 (scores/AV/
    out-proj matmuls in bf16): halves SBUF, enables the pipeline to fit.
  - K is normalized directly (like Q) instead of folding its norm into
    exp's scale: kills the end-of-phase-1 nkcols barrier + 128 tiny MMs.
  - rsqrt via Ln+Exp (one ACT table set with softmax's Exp - no table
    thrash); squares on GPSIMD not ACT; phase-3 PSUM drain on DVE.
  - Causal trim: scores/exp/AV instructions only cover i >= j_chunk
    (saves ~25k PE cycles + the zero-fill of P gaps).
  - attention processed in 512-wide i-quarters: PSUM fits proj(2) +
    scratch(2) + scores(2) + AV-accum(2) banks concurrently.
"""

import numpy as np
from contextlib import ExitStack

import concourse.bass as bass
import concourse.tile as tile
import concourse.mybir as mybir
from concourse import bacc
from concourse.hw_specs import get_activation_tables

F32 = mybir.dt.float32
BF16 = mybir.dt.bfloat16
MM_DT = mybir.dt.float32r
AF = mybir.ActivationFunctionType

D = 1024   # model dim
DH = 512   # head-group width per core (8 heads x 64)
NH = 8     # heads per core
HD = 64    # head dim
NKC = D // 128   # k-chunks over model dim
EPS = 1e-6


def _r(ap):
    return ap.bitcast(MM_DT)


def build_kernel(nc: bass.Bass, T: int = 2048):
    NTT = T // 512          # 512-wide t/i blocks
    NTS = T // 128          # 128-wide j chunks
    HW = min(1024, T)       # half width (q/k tile column span)
    NHALF = T // HW

    xt = nc.dram_tensor("xt", [D, T], BF16, kind="ExternalInput").ap()
    wqt = nc.dram_tensor("wqt", [D, DH], BF16, kind="ExternalInput").ap()
    wkt = nc.dram_tensor("wkt", [D, DH], BF16, kind="ExternalInput").ap()
    wvt = nc.dram_tensor("wvt", [D, DH], BF16, kind="ExternalInput").ap()
    wob = nc.dram_tensor("wob", [DH, D], BF16, kind="ExternalInput").ap()
    c2d = nc.dram_tensor("c2", [128, T], BF16, kind="ExternalInput").ap()
    s2d = nc.dram_tensor("s2", [128, T], BF16, kind="ExternalInput").ap()
    pswapd = nc.dram_tensor("pswap", [128, 128], F32, kind="ExternalInput").ap()
    bdiagd = nc.dram_tensor("bdiag", [128, 128], F32, kind="ExternalInput").ap()
    trid = nc.dram_tensor("tri", [128, 128], BF16, kind="ExternalInput").ap()
    yt = nc.dram_tensor("yt", [D, T], F32, kind="ExternalOutput").ap()

    # Pin the ACT table to the one set serving Ln AND Exp so bacc's
    # greedy table-load pass doesn't thrash between exp/ln sets.
    act_set_id = list(get_activation_tables(nc.m.arch)).index(
        "natural_log_exp_and_others")

    with tile.TileContext(nc) as tc, ExitStack() as ctx:
        # ---------------- pools (flat; all coexist) ---------------------
        cst = ctx.enter_context(tc.tile_pool(name="cst", bufs=1))
        qk_pool = ctx.enter_context(tc.tile_pool(name="qk", bufs=1))
        v_pool = ctx.enter_context(tc.tile_pool(name="v", bufs=1))
        o_pool = ctx.enter_context(tc.tile_pool(name="ot", bufs=1))
        w_pool = ctx.enter_context(tc.tile_pool(name="w", bufs=1))
        rc_pool = ctx.enter_context(tc.tile_pool(name="rc", bufs=1))
        x_pool = ctx.enter_context(tc.tile_pool(name="xs", bufs=5))
        sg_pool = ctx.enter_context(tc.tile_pool(name="sg", bufs=2))
        p_pool = ctx.enter_context(tc.tile_pool(name="pe", bufs=4))
        st_pool = ctx.enter_context(tc.tile_pool(name="st", bufs=3))
        e_pool = ctx.enter_context(tc.tile_pool(name="ep", bufs=2))
        # PSUM: 8 banks = scratch(2: proj/pswap/sumsq/oproj share one tag)
        # + scores(4: [128,1024] double-buffered) + AV accumulators(2)
        ps_scr = ctx.enter_context(tc.tile_pool(name="scr", bufs=2, space="PSUM"))
        ps_sc = ctx.enter_context(tc.tile_pool(name="sc", bufs=2, space="PSUM"))
        ps_ot = ctx.enter_context(tc.tile_pool(name="po", bufs=1, space="PSUM"))
        ps_pp = ps_scr  # alias kept for readability

        # ---------------- persistent tiles ------------------------------
        pswap = cst.tile([128, 128], F32, name="pswap_s")
        bdiag = cst.tile([128, 128], F32, name="bdiag_s")
        tri = cst.tile([128, 128], BF16, name="tri_s")
        epsb = cst.tile([128, 1], F32, name="epsb")
        # q/k: per head-pair-tile, per half: [128, HW] bf16 (normed+rotated)
        qt_s = [[qk_pool.tile([128, HW], BF16, name=f"qt{j}_{h}")
                 for h in range(NHALF)] for j in range(4)]
        kt_s = [[qk_pool.tile([128, HW], BF16, name=f"kt{j}_{h}")
                 for h in range(NHALF)] for j in range(4)]
        # V: [128, 8*128] bf16 per 128-token chunk. Per-head 128 cols =
        # [ones, 63 zeros, V 64] so the AV psum puts the softmax
        # denominator at row 0 and the payload at rows 64:128 - legal
        # base partitions (0 / 64) for the epilogue reads.
        v_s = [v_pool.tile([128, NH * 128], BF16, name=f"vt{j}")
               for j in range(NTS)]
        # attention out (normalized): 4 tiles [128, T] bf16
        otf = [o_pool.tile([128, T], BF16, name=f"otf{j}") for j in range(4)]
        wq_s = [w_pool.tile([128, DH], BF16, name=f"wq{k}") for k in range(NKC)]
        wk_s = [w_pool.tile([128, DH], BF16, name=f"wk{k}") for k in range(NKC)]
        wv_s = [w_pool.tile([128, DH], BF16, name=f"wv{k}") for k in range(NKC)]
        wo_s = [w_pool.tile([128, D], BF16, name=f"wo{k}") for k in range(4)]
        c2 = [rc_pool.tile([128, HW], BF16, name=f"c2_{h}")
              for h in range(NHALF)]
        s2 = [rc_pool.tile([128, HW], BF16, name=f"s2_{h}")
              for h in range(NHALF)]

        # ---------------- input DMAs ------------------------------------
        for k in range(NKC):
            ksl = slice(k * 128, (k + 1) * 128)
            nc.sync.dma_start(wq_s[k][:], wqt[ksl, :])
        nc.sync.dma_start(_r(pswap[:]), _r(pswapd[:]))
        nc.sync.dma_start(_r(bdiag[:]), _r(bdiagd[:]))
        nc.sync.dma_start(tri[:], trid[:])
        for k in range(NKC):
            ksl = slice(k * 128, (k + 1) * 128)
            nc.sync.dma_start(wv_s[k][:], wvt[ksl, :])
        for k4 in range(4):
            nc.sync.dma_start(wo_s[k4][:], wob[k4 * 128:(k4 + 1) * 128, :])
        # rotary tables for half 0 on the (initially idle) ACT queue
        nc.scalar.dma_start(c2[0][:], c2d[:, 0:HW])
        nc.scalar.dma_start(s2[0][:], s2d[:, 0:HW])
        nc.scalar.add_instruction(mybir.InstLoadActFuncSet(
            name=f"I-{nc.next_id()}", ins=[], outs=[],
            act_func_set_id=act_set_id))
        nc.gpsimd.memset(epsb[:], 8.0 * EPS)

        # ---------------- phase bodies ----------------------------------
        def proj_block(tt, jt, wsrc, dst_s):
            """Project + rotate + RMS-normalize one [128, 512] q/k block."""
            w0 = tt < 2  # before attention starts: borrow its idle PSUM
            half, loc = (tt * 512) // HW, (tt * 512) % HW
            lsl = slice(loc, loc + 512)
            pp = (ps_sc if w0 else ps_scr).tile(
                [128, 512], F32, name="pp", tag="sc" if w0 else "scr")
            jsl = slice(jt * 128, (jt + 1) * 128)
            for k in range(NKC):
                nc.tensor.matmul(pp[:], wsrc[k][:, jsl], xc_s[k][:],
                                 start=(k == 0), stop=(k == NKC - 1))
            qw = sg_pool.tile([128, 512], F32, name="qw", tag="qw")
            nc.vector.tensor_copy(_r(qw[:]), pp[:])
            # partition half-swap (d <-> d^32) for the rotary cross term
            xs = (ps_ot if w0 else ps_scr).tile(
                [128, 512], F32, name="xs", tag="po0" if w0 else "scr")
            nc.tensor.matmul(xs[:], _r(pswap[:]), _r(qw[:]),
                             start=True, stop=True)
            # sumsq over each head's 64 dims (rotation preserves norms)
            sq = sg_pool.tile([128, 512], F32, name="sq", tag="sq")
            nc.gpsimd.tensor_mul(_r(sq[:]), qw[:], qw[:])
            ms = (ps_ot if w0 else ps_scr).tile(
                [128, 512], F32, name="ms", tag="po1" if w0 else "scr")
            nc.tensor.matmul(ms[:], _r(bdiag[:]), _r(sq[:]),
                             start=True, stop=True)
            # s1 = (sumsq/8 + 8eps)^-1/2 via exp(-0.5*ln(x)): same ACT
            # table set as softmax's Exp (no table switching)
            s1 = sg_pool.tile([128, 512], F32, name="s1", tag="s1")
            nc.scalar.activation(s1[:], ms[:], AF.Ln, scale=0.125, bias=epsb[:])
            nc.scalar.activation(s1[:], s1[:], AF.Exp, scale=-0.5)
            # rotary: q' = q*c2 + swap(q)*s2, then scale by s1, cast bf16
            nc.gpsimd.tensor_mul(_r(qw[:]), qw[:], c2[half][:, lsl])
            xsb = sg_pool.tile([128, 512], F32, name="xsb", tag="xsb")
            nc.vector.tensor_mul(xsb[:], xs[:], s2[half][:, lsl])
            nc.gpsimd.tensor_add(_r(qw[:]), qw[:], xsb[:])
            nc.gpsimd.tensor_mul(dst_s[jt][half][:, lsl], qw[:], s1[:])

        def v_block(tt, ts):
            ci = tt * 4 + ts
            pv = ps_pp.tile([128, 512], F32, name="pv", tag="scr")
            for k in range(NKC):
                nc.tensor.matmul(
                    pv[:], xc_s[k][:, ts * 128:(ts + 1) * 128],
                    wv_s[k][:], start=(k == 0), stop=(k == NKC - 1))
            v3 = v_s[ci].rearrange("p (h e) -> p h e", h=NH)
            nc.vector.tensor_copy(v3[:, :, 64:128],
                                  pv.rearrange("p (h e) -> p h e", h=NH))

        xc_s = None

        def tt_block(tt):
            nonlocal xc_s
            # two 128-row k-chunks per tile/DMA: halves the SWDGE holds
            # on the Pool queue (its compute was getting starved)
            xc_s = []
            for k2 in range(NKC // 2):
                xc = x_pool.tile([128, 2, 512], BF16, name="xc", tag="xc")
                src_ap = xt[k2 * 256:(k2 + 1) * 256,
                            tt * 512:(tt + 1) * 512]
                nc.gpsimd.dma_start(
                    xc[:],
                    src_ap.rearrange("(a p) f -> p a f", a=2))
                xc_s.append(xc[:, 0, :])
                xc_s.append(xc[:, 1, :])
            if tt == 0:
                # wk rides the Pool DMA queue behind tt0's x chunks: it
                # lands right when tt0's K projections need it, while the
                # SP queue carries wq/wv/wo in parallel.
                for k in range(NKC):
                    nc.gpsimd.dma_start(
                        wk_s[k][:], wkt[k * 128:(k + 1) * 128, :])
                    # (wk stays on the Pool queue: it overlaps the SP
                    # queue's wq/wv stream during the cold ramp)
            for jt in range(4):
                proj_block(tt, jt, wq_s, qt_s)
            for jt in range(4):
                proj_block(tt, jt, wk_s, kt_s)
            for ts in range(4):
                v_block(tt, ts)

        def attn_quarter(iq):
            """Causal attention for i in [512*iq, 512*(iq+1)), all 8 heads."""
            ih, iloc = (iq * 512) // HW, (iq * 512) % HW
            nj = 4 * iq + 4
            for hp in range(4):
                ops = {}
                for h2 in range(2):
                    ops[h2] = ps_ot.tile([128, 512], F32, name="otp",
                                         tag=f"po{h2}")
                for jt in range(nj):
                    jh, jloc = (jt * 128) // HW, (jt * 128) % HW
                    jsl = slice(jloc, jloc + 128)
                    off0 = max(0, 128 * (jt - 4 * iq))  # diag offset
                    sc = ps_sc.tile([128, 1024], F32, name="sc", tag="sc")
                    for h2 in range(2):
                        ho = h2 * 64
                        nc.tensor.matmul(
                            sc[:, h2 * 512 + off0:h2 * 512 + 512],
                            kt_s[hp][jh][ho:ho + 64, jsl],
                            qt_s[hp][ih][ho:ho + 64,
                                         iloc + off0:iloc + 512],
                            start=True, stop=True)
                    # one exp instruction covers both heads (strided AP)
                    p = p_pool.tile([128, 1024], BF16, name="p", tag="p")
                    sc3 = sc.rearrange("p (g c) -> p g c", g=2)
                    p3 = p.rearrange("p (g c) -> p g c", g=2)
                    nc.scalar.activation(p3[:, :, off0:512],
                                         sc3[:, :, off0:512], AF.Exp)
                    if jt >= 4 * iq:  # diagonal chunk: causal mask
                        for h2 in range(2):
                            nc.gpsimd.tensor_mul(
                                p[:, h2 * 512 + off0:h2 * 512 + off0 + 128],
                                p[:, h2 * 512 + off0:h2 * 512 + off0 + 128],
                                tri[:])
                    for h2 in range(2):
                        h = 2 * hp + h2
                        nc.tensor.matmul(
                            ops[h2][:, off0:512],
                            v_s[jt][:, 128 * h:128 * h + 128],
                            p[:, h2 * 512 + off0:h2 * 512 + 512],
                            start=(jt == 0), stop=(jt == nj - 1))
                # epilogue: rows 1..64 / row 0 (softmax denominator).
                # One fast copy drains the PSUM accumulator so the next
                # head-pair's AV chain can start while we divide from SBUF.
                for h2 in range(2):
                    ho = h2 * 64
                    rden = e_pool.tile([1, 512], F32, name="rden", tag="rden")
                    nc.vector.reciprocal_approx_fast(out=rden[:],
                                                     in_=ops[h2][0:1, :])
                    rb = e_pool.tile([64, 512], F32, name="rb", tag="rb")
                    nc.gpsimd.partition_broadcast(rb[:], rden[:], channels=64)
                    nc.vector.tensor_mul(
                        otf[hp][ho:ho + 64, iq * 512:(iq + 1) * 512],
                        ops[h2][64:128, :], rb[:])

        def oproj_block(tt):
            tsl = slice(tt * 512, (tt + 1) * 512)
            for dt_ in range(8):
                dsl = slice(dt_ * 128, (dt_ + 1) * 128)
                py = ps_scr.tile([128, 512], F32, name="py", tag="scr")
                for k4 in range(4):
                    nc.tensor.matmul(py[:], wo_s[k4][:, dsl],
                                     otf[k4][:, tsl],
                                     start=(k4 == 0), stop=(k4 == 3))
                st = st_pool.tile([128, 512], F32, name="st", tag="st")
                nc.vector.tensor_copy(st[:], py[:])
                nc.sync.dma_start(yt[dsl, tsl], st[:])

        # ---------------- emission (pipelined program order) ------------
        # V ones/zero columns on the DVE, which idles until ~7us
        for j in range(NTS):
            v3 = v_s[j].rearrange("p (h e) -> p h e", h=NH)
            nc.vector.memset(v3[:, :, 0:1], 1.0)
            nc.vector.memset(v3[:, :, 1:64], 0.0)
        tt_block(0)
        attn_quarter(0)       # needs only tt0 (i < 512, j < 512)
        if NTT > 1:
            tt_block(1)
        if NTT > 2:
            nc.sync.dma_start(c2[1][:], c2d[:, HW:2 * HW])
            nc.sync.dma_start(s2[1][:], s2d[:, HW:2 * HW])
            attn_quarter(1)
            tt_block(2)
            attn_quarter(2)   # needs only tt2's q + j-chunks < 1536
            oproj_block(0)
            tt_block(3)
            attn_quarter(3)
            oproj_block(1)
            oproj_block(2)
            oproj_block(3)
        elif NTT > 1:
            attn_quarter(1)
            oproj_block(0)
            oproj_block(1)
        else:
            oproj_block(0)
    return nc


# ---------------- host-side tables & shard prep -------------------------

def host_tables(T: int = 2048):
    import ml_dtypes
    n = HD // 4
    af = (1.0 / 1024) ** np.linspace(0, 1, n, dtype=np.float32)
    af = np.concatenate([af, np.zeros(n, np.float32)])  # [32]
    theta = np.outer(np.arange(T, dtype=np.float32), af)  # [T, 32]
    cosT = np.cos(theta).T.astype(np.float32)  # [32, T]
    sinT = np.sin(theta).T.astype(np.float32)
    c2 = np.tile(cosT, (4, 1))                              # [128, T]
    s2 = np.tile(np.concatenate([sinT, -sinT], 0), (2, 1))  # [128, T]
    km = np.arange(128)
    pswap = (km[:, None] == (km[None, :] ^ 32)).astype(np.float32)
    bdiag = ((km[:, None] // 64) == (km[None, :] // 64)).astype(np.float32)
    r_ = np.arange(128)[:, None]
    c_ = np.arange(128)[None, :]
    tri = (c_ >= r_).astype(ml_dtypes.bfloat16)
    return {"c2": np.ascontiguousarray(c2.astype(ml_dtypes.bfloat16)),
            "s2": np.ascontiguousarray(s2.astype(ml_dtypes.bfloat16)),
            "pswap": pswap, "bdiag": bdiag, "tri": tri}


def core_inputs(x, wq, wk, wv, wo, core: int, T: int = 2048):
    import ml_dtypes
    b, g = core % 4, core // 4
    sl = slice(g * DH, (g + 1) * DH)
    m = {
        "xt": np.ascontiguousarray(
            np.asarray(x[b]).T.astype(ml_dtypes.bfloat16)),
        "wqt": np.ascontiguousarray(
            np.asarray(wq)[sl, :].T.astype(ml_dtypes.bfloat16)),
        "wkt": np.ascontiguousarray(
            np.asarray(wk)[sl, :].T.astype(ml_dtypes.bfloat16)),
        "wvt": np.ascontiguousarray(
            np.asarray(wv)[sl, :].T.astype(ml_dtypes.bfloat16)),
        "wob": np.ascontiguousarray(
            np.asarray(wo)[:, sl].T.astype(ml_dtypes.bfloat16)),
    }
    m.update(host_tables(T))
    return m


_CACHE = {}


def _get_nc(T: int = 2048):
    key = ("nc", T)
    if key not in _CACHE:
        nc = bacc.Bacc("TRN2", target_bir_lowering=False, debug=False)
        build_kernel(nc, T)
        nc.compile()
        _CACHE[key] = nc
    return _CACHE[key]


def kernel(x, wq, wk, wv, wo, mask=None):
    from concourse import bass_utils
    nc = _get_nc(2048)
    in_maps = [core_inputs(x, wq, wk, wv, wo, c) for c in range(8)]
    res = bass_utils.run_bass_kernel_spmd(nc, in_maps, list(range(8)))
    outs = [np.asarray(res.results[c]["yt"]) for c in range(8)]
    out = np.empty((4, 2048, 1024), np.float32)
    for b in range(4):
        out[b] = (outs[b] + outs[b + 4]).T
    return out
